# revision 10
# baseline (speedup 1.0000x reference)
"""Trainium2 Bass kernel for nn_DMRGCNGPGraph (DMRGCN + group-partitioned graphs).

Contract: kernel(**inputs) takes the FULL inputs from setup_inputs() and
returns the full (pred, gi) outputs. Internally: data-parallel over B across
8 NeuronCores (scene b -> core b).

Numerics:
  - group assignment: exact (fp32 distance compare, boolean reachability via
    4 bf16 0/1 matrix squarings == 16 min-propagation iterations).
  - A_inter: exact (0/1 bf16 matmuls, binarized between products).
  - backbone matmuls: float32r (RNE to 11 explicit mantissa bits) ->
    end-to-end scale-relative error ~5e-4 vs the fp32 reference.
"""
import sys

sys.path.insert(0, "/opt/trn_rl_repo")

from contextlib import ExitStack

import numpy as np

import concourse.bass as bass
import concourse.tile as tile
from concourse import bacc, mybir
from concourse.bass_utils import run_bass_kernel_spmd

FP32 = mybir.dt.float32
F32R = mybir.dt.float32r
BF16 = mybir.dt.bfloat16
I32 = mybir.dt.int32
AF = mybir.ActivationFunctionType
OP = mybir.AluOpType
AX = mybir.AxisListType

B, T, N, R = 8, 8, 256, 3
DIMS = [2, 64, 64, 64, 64, 5]
NL = 5
PRED_LEN = 12
SCALES = (0.5, 1.0, 2.0)
GROUP_TH2 = 4.0  # 2.0**2
NCHUNK = 2  # N / 128
TN = T * N  # 2048


def _build_program():
    nc = bacc.Bacc("TRN2", target_bir_lowering=False, debug=False, num_devices=8)

    # ---- DRAM parameters (per-core views) ----
    V_p = nc.declare_dram_parameter("V", [T, N, 2], FP32, isOutput=False)
    A_p = nc.declare_dram_parameter("A", [R, T, N, N], FP32, isOutput=False)
    w_ps = [
        nc.declare_dram_parameter(f"w{l}", [DIMS[l], 256], FP32, isOutput=False)
        for l in range(NL)
    ]  # W_l padded: col = r*cout+d, zero-padded to 256
    k_ps = [
        nc.declare_dram_parameter(f"k{l}", [64, 192], FP32, isOutput=False)
        for l in range(NL - 1)
    ]  # col = dt*64 + cout ; value = tk[l][cout, cin, dt, 0] (cin on partitions)
    wt_p = nc.declare_dram_parameter("wt", [T, PRED_LEN], FP32, isOutput=False)
    iota_p = nc.declare_dram_parameter("iota", [1, N], FP32, isOutput=False)

    pred_p = nc.declare_dram_parameter("pred", [5, PRED_LEN, N], FP32, isOutput=True)
    gi_p = nc.declare_dram_parameter("gi", [N], I32, isOutput=True)
    lab_bounce = nc.dram_tensor("lab_bounce", [N], FP32)
    fused_bounce = nc.dram_tensor("fused_bounce", [5, TN], FP32)

    with tile.TileContext(nc) as tc, ExitStack() as ctx:
        big = ctx.enter_context(tc.tile_pool(name="big", bufs=1))
        wpool = ctx.enter_context(tc.tile_pool(name="wpool", bufs=1))
        grp = ctx.enter_context(tc.tile_pool(name="grp", bufs=1))
        act = ctx.enter_context(tc.tile_pool(name="act", bufs=1))
        tmp = ctx.enter_context(tc.tile_pool(name="tmp", bufs=2))
        ps_proj = ctx.enter_context(tc.tile_pool(name="ps_proj", bufs=2, space="PSUM"))
        ps_agg = ctx.enter_context(tc.tile_pool(name="ps_agg", bufs=2, space="PSUM"))
        ps_grp = ctx.enter_context(tc.tile_pool(name="ps_grp", bufs=3, space="PSUM"))

        # =========== load raw A (fp32), big resident tiles ===========
        # araw[c]: (128, R*T*N) ; col = (r*T + t)*N + m ; partition = n in chunk c
        araw = [big.tile([128, R * T * N], FP32, tag=f"araw{c}", name=f"araw{c}") for c in range(NCHUNK)]
        for c in range(NCHUNK):
            for r in range(R):
                nc.sync.dma_start(
                    araw[c][:, r * TN : (r + 1) * TN].rearrange(
                        "p (t m) -> p t m", t=T
                    ),
                    A_p[r, :, c * 128 : (c + 1) * 128, :].rearrange("t p m -> p t m"),
                )

        # =========== weights ===========
        wks = []
        for l in range(NL):
            wf = wpool.tile([DIMS[l], 256], FP32, tag=f"wf{l}", name=f"wf{l}")
            nc.sync.dma_start(wf[:], w_ps[l][:])
            wr = wpool.tile([DIMS[l], 256], F32R, tag=f"wr{l}", name=f"wr{l}")
            nc.vector.tensor_copy(wr[:], wf[:])
            wks.append(wr)
        kts = []
        for l in range(NL - 1):
            kf = wpool.tile([64, 192], FP32, tag=f"kf{l}", name=f"kf{l}")
            nc.sync.dma_start(kf[:], k_ps[l][:])
            kr = wpool.tile([64, 192], F32R, tag=f"kr{l}", name=f"kr{l}")
            nc.vector.tensor_copy(kr[:], kf[:])
            kts.append(kr)
        wtf = wpool.tile([T, PRED_LEN], FP32)
        nc.sync.dma_start(wtf[:], wt_p[:])
        wtr = wpool.tile([T, PRED_LEN], F32R)
        nc.vector.tensor_copy(wtr[:], wtf[:])

        iota_row = wpool.tile([1, N], FP32)
        nc.sync.dma_start(iota_row[:], iota_p[:])
        iota_bc = wpool.tile([128, N], FP32)
        nc.gpsimd.partition_broadcast(iota_bc[:], iota_row[:])

        # x0 = V transposed to (2, T*N), rounded
        x0f = tmp.tile([2, TN], FP32, bufs=1)
        nc.sync.dma_start(x0f[:], V_p.rearrange("t n c -> c (t n)"))
        x0 = act.tile([2, TN], F32R)
        nc.vector.tensor_copy(x0[:], x0f[:])

        # =========== grouping ===========
        # last positions: row (1,256) and per-chunk col (128,1) for x and y coord
        lx_row = grp.tile([1, N], FP32)
        ly_row = grp.tile([1, N], FP32)
        nc.sync.dma_start(lx_row[:], V_p[T - 1, :, 0])
        nc.sync.dma_start(ly_row[:], V_p[T - 1, :, 1])
        lx_bc = grp.tile([128, N], FP32)
        ly_bc = grp.tile([128, N], FP32)
        nc.gpsimd.partition_broadcast(lx_bc[:], lx_row[:])
        nc.gpsimd.partition_broadcast(ly_bc[:], ly_row[:])

        adjb = []  # bf16 0/1 adjacency per chunk
        for c in range(NCHUNK):
            lx_col = grp.tile([128, 1], FP32, tag=f"lxc{c}", name=f"lxc{c}")
            ly_col = grp.tile([128, 1], FP32, tag=f"lyc{c}", name=f"lyc{c}")
            nc.sync.dma_start(
                lx_col[:], V_p[T - 1, c * 128 : (c + 1) * 128, 0]
            )
            nc.sync.dma_start(
                ly_col[:], V_p[T - 1, c * 128 : (c + 1) * 128, 1]
            )
            dx = tmp.tile([128, N], FP32, tag="gdx", bufs=1)
            nc.vector.tensor_scalar(dx[:], lx_bc[:], lx_col[:], None, OP.subtract)
            dy = tmp.tile([128, N], FP32, tag="gdy", bufs=1)
            nc.vector.tensor_scalar(dy[:], ly_bc[:], ly_col[:], None, OP.subtract)
            d2 = tmp.tile([128, N], FP32, tag="gd2", bufs=1)
            nc.vector.tensor_tensor(d2[:], dx[:], dx[:], OP.mult)
            dy2 = tmp.tile([128, N], FP32, tag="gdy2", bufs=1)
            nc.vector.tensor_tensor(dy2[:], dy[:], dy[:], OP.mult)
            nc.vector.tensor_tensor(d2[:], d2[:], dy2[:], OP.add)
            ab = grp.tile([128, N], BF16, tag=f"adjb{c}", name=f"adjb{c}")
            nc.vector.tensor_scalar(ab[:], d2[:], GROUP_TH2, None, OP.is_le)
            adjb.append(ab)

        # 4 squarings: B^2, B^4, B^8, B^16 (symmetric, 0/1 exact)
        cur = adjb
        for sq in range(4):
            last = sq == 3
            nxt = []
            for c in range(NCHUNK):
                ps = ps_grp.tile([128, N], FP32, tag="grp")
                for jc in range(NCHUNK):
                    nc.tensor.matmul(
                        ps[:],
                        cur[jc][:, c * 128 : (c + 1) * 128],
                        cur[jc][:],
                        start=(jc == 0),
                        stop=(jc == NCHUNK - 1),
                    )
                ob = grp.tile([128, N], FP32 if last else BF16, tag=f"b{2 ** (sq + 1)}_{c}", name=f"b{2 ** (sq + 1)}_{c}")
                nc.vector.tensor_scalar(ob[:], ps[:], 0.0, None, OP.is_gt)
                nxt.append(ob)
            cur = nxt
        b16 = cur  # fp32 0/1 reach matrix chunks

        # label = min reachable index: masked = (iota-256)*b16 + 256 ; min over free
        lab_col = []
        for c in range(NCHUNK):
            mk = tmp.tile([128, N], FP32, tag="gmask", bufs=1)
            nc.vector.scalar_tensor_tensor(
                mk[:], iota_bc[:], float(N), b16[c][:], OP.subtract, OP.mult
            )
            nc.vector.tensor_scalar(mk[:], mk[:], float(N), None, OP.add)
            lc = grp.tile([128, 1], FP32, tag=f"lab{c}", name=f"lab{c}")
            nc.vector.tensor_reduce(lc[:], mk[:], AX.X, OP.min)
            lab_col.append(lc)
            # gi output
            li = tmp.tile([128, 1], I32, tag="labi", bufs=1)
            nc.vector.tensor_copy(li[:], lc[:])
            nc.sync.dma_start(gi_p[c * 128 : (c + 1) * 128], li[:])
            nc.sync.dma_start(lab_bounce[c * 128 : (c + 1) * 128], lc[:])

        lab_row = grp.tile([1, N], FP32)
        nc.sync.dma_start(lab_row[:], lab_bounce[:])
        lab_bc = grp.tile([128, N], FP32)
        nc.gpsimd.partition_broadcast(lab_bc[:], lab_row[:])

        same_f = []  # fp32 0/1 per chunk
        same_b = []  # bf16 0/1 per chunk (for S matmuls)
        nots = []  # fp32 0/1 (~same)
        for c in range(NCHUNK):
            sf = grp.tile([128, N], FP32, tag=f"same{c}", name=f"same{c}")
            nc.vector.tensor_scalar(sf[:], lab_bc[:], lab_col[c][:], None, OP.is_equal)
            same_f.append(sf)
            sb = grp.tile([128, N], BF16, tag=f"sameb{c}", name=f"sameb{c}")
            nc.vector.tensor_copy(sb[:], sf[:])
            same_b.append(sb)
            nf = grp.tile([128, N], FP32, tag=f"nots{c}", name=f"nots{c}")
            nc.vector.tensor_scalar(nf[:], lab_bc[:], lab_col[c][:], None, OP.not_equal)
            nots.append(nf)

        # =========== helper: backbone over one path's A ===========
        fused = act.tile([5, TN], FP32)

        def backbone(apath, path_idx):
            """apath[c]: (128, R*T*N) f32r tile, col (r*T+t)*N + m."""
            x = x0
            for l in range(NL):
                cin, cout = DIMS[l], DIMS[l + 1]
                # ---- projection: h[c][n, t*256 + r*cout+d] ----
                h = [
                    tmp.tile([128, TN], F32R, tag=f"h{c}", name=f"h{c}", bufs=1)
                    for c in range(NCHUNK)
                ]
                for c in range(NCHUNK):
                    for t0 in range(0, T, 2):
                        pp = ps_proj.tile([128, 512], FP32, tag="proj")
                        for dt in range(2):
                            t = t0 + dt
                            nc.tensor.matmul(
                                pp[:, dt * 256 : (dt + 1) * 256],
                                x[:, t * N + c * 128 : t * N + c * 128 + 128],
                                wks[l][:],
                                start=True,
                                stop=True,
                            )
                        nc.vector.tensor_copy(
                            h[c][:, t0 * N : (t0 + 2) * N], pp[:]
                        )
                # ---- aggregation + (relu | fused-accum) ----
                if l < NL - 1:
                    xn = act.tile([64, TN], F32R, tag="xrelu")
                for t in range(T):
                    pa = ps_agg.tile([cout, N], FP32, tag="agg")
                    k = 0
                    for r in range(R):
                        for c in range(NCHUNK):
                            nc.tensor.matmul(
                                pa[:],
                                h[c][:, t * N + r * cout : t * N + r * cout + cout],
                                apath[c][:, (r * T + t) * N : (r * T + t + 1) * N],
                                start=(k == 0),
                                stop=(k == 2 * R - 1),
                            )
                            k += 1
                    if l < NL - 1:
                        nc.scalar.activation(
                            xn[:, t * N : (t + 1) * N], pa[:], AF.Relu
                        )
                    elif path_idx == 0:
                        nc.vector.tensor_copy(fused[:, t * N : (t + 1) * N], pa[:])
                    else:
                        nc.vector.tensor_tensor(
                            fused[:, t * N : (t + 1) * N],
                            fused[:, t * N : (t + 1) * N],
                            pa[:],
                            OP.add,
                        )
                if l == NL - 1:
                    return
                # ---- temporal conv (3,1) SAME ----
                xc = act.tile([64, TN], F32R, tag="xconv")
                for t in range(T):
                    pc = ps_agg.tile([64, N], FP32, tag="agg")
                    dts = [d for d in (-1, 0, 1) if 0 <= t + d < T]
                    for i, d in enumerate(dts):
                        nc.tensor.matmul(
                            pc[:],
                            kts[l][:, (d + 1) * 64 : (d + 2) * 64],
                            xn[:, (t + d) * N : (t + d + 1) * N],
                            start=(i == 0),
                            stop=(i == len(dts) - 1),
                        )
                    nc.scalar.activation(xc[:, t * N : (t + 1) * N], pc[:], AF.Copy)
                x = xc

        # =========== path 1: agent (thresholded distance relation) ===========
        apath = [big.tile([128, R * T * N], F32R, tag=f"apath{c}", name=f"apath{c}") for c in range(NCHUNK)]
        for c in range(NCHUNK):
            for si, s in enumerate(SCALES):
                # in: araw[c] r=1 block (T*N wide); out: apath block si
                nc.vector.scalar_tensor_tensor(
                    apath[c][:, si * TN : (si + 1) * TN],
                    araw[c][:, 1 * TN : 2 * TN],
                    float(s),
                    araw[c][:, 1 * TN : 2 * TN],
                    OP.is_le,
                    OP.mult,
                )
        backbone(apath, 0)

        # =========== path 2: intra (A * same) ===========
        for c in range(NCHUNK):
            for r in range(R):
                for t in range(T):
                    nc.vector.tensor_tensor(
                        apath[c][:, (r * T + t) * N : (r * T + t + 1) * N],
                        araw[c][:, (r * T + t) * N : (r * T + t + 1) * N],
                        same_f[c][:],
                        OP.mult,
                    )
        backbone(apath, 1)

        # =========== path 3: inter ===========
        # crossb = (A>0)*(~same)  (bf16 0/1), built per (r,t)
        for r in range(R):
            for t in range(T):
                crossb = [tmp.tile([128, N], BF16, tag=f"crossb{c}", name=f"crossb{c}") for c in range(NCHUNK)]
                for c in range(NCHUNK):
                    nc.vector.scalar_tensor_tensor(
                        crossb[c][:],
                        araw[c][:, (r * T + t) * N : (r * T + t + 1) * N],
                        0.0,
                        nots[c][:],
                        OP.is_gt,
                        OP.mult,
                    )
                # P1 = (S @ cross) > 0 ; A_inter = (P1 @ S) > 0
                p1b = [tmp.tile([128, N], BF16, tag=f"p1b{c}", name=f"p1b{c}") for c in range(NCHUNK)]
                for c in range(NCHUNK):
                    ps = ps_grp.tile([128, N], FP32, tag="grp")
                    for jc in range(NCHUNK):
                        nc.tensor.matmul(
                            ps[:],
                            same_b[jc][:, c * 128 : (c + 1) * 128],
                            crossb[jc][:],
                            start=(jc == 0),
                            stop=(jc == NCHUNK - 1),
                        )
                    nc.vector.tensor_scalar(p1b[c][:], ps[:], 0.0, None, OP.is_gt)
                for c in range(NCHUNK):
                    ps = ps_grp.tile([128, N], FP32, tag="grp")
                    for jc in range(NCHUNK):
                        nc.tensor.matmul(
                            ps[:],
                            p1b[jc][:, c * 128 : (c + 1) * 128],
                            same_b[jc][:],
                            start=(jc == 0),
                            stop=(jc == NCHUNK - 1),
                        )
                    nc.vector.tensor_scalar(
                        apath[c][:, (r * T + t) * N : (r * T + t + 1) * N],
                        ps[:],
                        0.0,
                        None,
                        OP.is_gt,
                    )
        backbone(apath, 2)

        # =========== head: pred = einsum('ctn,tp->cpn', fused/3, wt) ===========
        nc.sync.dma_start(fused_bounce[:], fused[:])
        fusedT = act.tile([T, 5 * N], FP32, tag="xrelu")
        nc.sync.dma_start(
            fusedT[:].rearrange("t (c n) -> t c n", c=5),
            fused_bounce.rearrange("c (t n) -> t c n", t=T),
        )
        fusedTr = act.tile([T, 5 * N], F32R, tag="xconv")
        nc.vector.tensor_scalar(fusedTr[:], fusedT[:], 1.0 / 3.0, None, OP.mult)
        head_sb = act.tile([PRED_LEN, 5 * N], FP32, tag="x0")
        for k, (c0, cw) in enumerate(((0, 2), (2, 2), (4, 1))):
            ph = ps_agg.tile([PRED_LEN, cw * N], FP32, tag="agg")
            nc.tensor.matmul(
                ph[:],
                wtr[:],
                fusedTr[:, c0 * N : (c0 + cw) * N],
                start=True,
                stop=True,
            )
            nc.vector.tensor_copy(head_sb[:, c0 * N : (c0 + cw) * N], ph[:])
        nc.sync.dma_start(
            pred_p.rearrange("c p n -> p c n"),
            head_sb[:].rearrange("p (c n) -> p c n", n=N),
        )

    nc.compile()
    return nc


_NC = None


def _round_f32r(x):
    xb = np.ascontiguousarray(x, np.float32).view(np.uint32).astype(np.uint64)
    lsb = (xb >> 12) & 1
    r = (xb + 0x7FF + lsb) & ~np.uint64(0xFFF)
    return r.astype(np.uint32).view(np.float32)


def kernel(V_obs, A_obs, params):
    global _NC
    if _NC is None:
        _NC = _build_program()
    nc = _NC

    V_obs = np.asarray(V_obs, np.float32)
    A_obs = np.asarray(A_obs, np.float32)
    w = [np.asarray(x, np.float32) for x in params["w"]]
    tk = [np.asarray(x, np.float32) for x in params["tk"]]
    wt = np.asarray(params["wt"], np.float32)

    shared = {}
    for l in range(NL):
        cin, cout = DIMS[l], DIMS[l + 1]
        wc = np.zeros((cin, 256), np.float32)
        wc[:, : R * cout] = w[l].transpose(1, 0, 2).reshape(cin, R * cout)
        shared[f"w{l}"] = wc
    for l in range(NL - 1):
        kc = np.zeros((64, 192), np.float32)
        for dt in range(3):
            kc[:, dt * 64 : (dt + 1) * 64] = tk[l][:, :, dt, 0].T
        shared[f"k{l}"] = kc
    shared["wt"] = wt
    shared["iota"] = np.arange(N, dtype=np.float32).reshape(1, N)

    in_maps = [
        dict(shared, V=np.ascontiguousarray(V_obs[b]), A=np.ascontiguousarray(A_obs[b]))
        for b in range(B)
    ]
    res = run_bass_kernel_spmd(nc, in_maps, list(range(B)))
    pred = np.stack([res.results[b]["pred"] for b in range(B)])
    gi = np.stack([res.results[b]["gi"] for b in range(B)])
    return pred.astype(np.float32), gi.astype(np.int32)
